# revision 1
# baseline (speedup 1.0000x reference)
"""Trainium2 Bass kernel for nn_Attention_4183298146960.

GQA causal attention layer: B=2, S=2048, HIDDEN=2048, 16 q heads / 4 kv heads,
head_dim=128, RoPE (interleaved pairs), causal softmax, output projection.

Sharding (8 cores, SPMD-uniform program):
  core c owns q heads {2c, 2c+1} and kv head c//2, for BOTH batches
  (tokens axis = [batch0 | batch1] = 4096).  QKV + RoPE + attention are
  fully local; the output projection needs all 16 heads' features, so the
  per-core attention outputs [256 feat, 4096 tok] are AllGathered (8-core
  mesh, chunked over q-tile slabs processed in reverse causal order so the
  gathers overlap attention compute), after which each core computes its
  256 output columns of W_o.

Layouts on device (partition dim first):
  feature-major qT/kT [head_dim, tokens] for scores; token-major v
  [tokens, head_dim] for PV; scores computed transposed [k, q] so softmax
  needs no max-subtraction (scores are O(+-10), exp is fp32-safe) and the
  denominator is a ones-matrix matmul producing the broadcast denominator
  directly; probabilities stay unnormalized until after PV.
  RoPE head dims are permuted [even | odd] via host-side W row permutation
  so the rotation is a 64-partition swap (SBUF->SBUF DMA) + DVE ops.
"""

import numpy as np
import ml_dtypes

import concourse.bass as bass
import concourse.mybir as mybir
import concourse.tile as tile
from concourse import bacc
from concourse.bass_utils import run_bass_kernel_spmd

BF16 = ml_dtypes.bfloat16

HEADS = 16
KV_HEADS = 4
HIDDEN = 2048
HD = 128
S = 2048
B = 2
T = B * S                      # 4096 token axis (both batches)
HT = HIDDEN // 128             # 16 hidden tiles
NQT = S // 512                 # 4 q-tiles of 512 per batch
SCALE = 1.0 / float(np.sqrt(HD))
RG8 = [[0, 1, 2, 3, 4, 5, 6, 7]]
QT_ORDER = [0, 1, 2, 3]        # biggest chunk last: its att hides the prior AGs

_COMPILED = None


def _build():
    dt = mybir.dt
    nc = bacc.Bacc("TRN2", target_bir_lowering=False, debug=False, num_devices=8)

    xT = nc.dram_tensor("xT", [128, HT, T], dt.bfloat16, kind="ExternalInput")
    wqk = nc.dram_tensor("wqk", [128, HT, 384], dt.bfloat16, kind="ExternalInput")
    wv = nc.dram_tensor("wv", [128, HT, 128], dt.bfloat16, kind="ExternalInput")
    wo = nc.dram_tensor("wo", [128, HT, 256], dt.bfloat16, kind="ExternalInput")
    cc = nc.dram_tensor("cc", [128, T], dt.bfloat16, kind="ExternalInput")
    ss = nc.dram_tensor("ss", [128, T], dt.bfloat16, kind="ExternalInput")
    msk = nc.dram_tensor("msk", [128, 4 * 512], dt.bfloat16, kind="ExternalInput")
    ones128 = nc.dram_tensor("ones128", [128, 128], dt.bfloat16, kind="ExternalInput")
    outT = nc.dram_tensor("outT", [256, T], dt.float32, kind="ExternalOutput")

    mult = mybir.AluOpType.mult
    add = mybir.AluOpType.add
    divide = mybir.AluOpType.divide
    Exp = mybir.ActivationFunctionType.Exp

    with tile.TileContext(nc) as tc:
        with (
            tc.tile_pool(name="const", bufs=1) as constp,
            tc.tile_pool(name="dram", bufs=1, space="DRAM") as dram,
        ):
            qcat = constp.tile([128, 2 * T], dt.bfloat16)   # 2 local q heads
            kT = constp.tile([128, T], dt.bfloat16)
            vsb = constp.tile([128, T], dt.bfloat16)        # token-major v tiles
            wo_sb = constp.tile([128, HT, 256], dt.bfloat16)
            msk_sb = constp.tile([128, 4 * 512], dt.bfloat16)
            ones_sb = constp.tile([128, 128], dt.bfloat16)

            # ---------------- QKV + RoPE ----------------
            with (
                tc.tile_pool(name="qkvw", bufs=1) as qkvw,
                tc.tile_pool(name="xp", bufs=2) as xp,
                tc.tile_pool(name="rp", bufs=4) as rp,
                tc.tile_pool(name="qkps", bufs=2, space="PSUM") as qkps,
                tc.tile_pool(name="vps", bufs=2, space="PSUM") as vps,
            ):
                wqk_sb = qkvw.tile([128, HT, 384], dt.bfloat16)
                for hq in range(4):   # split: first MMs start after 1/4 arrives
                    nc.sync.dma_start(
                        wqk_sb[:, hq * 4:(hq + 1) * 4, :], wqk[:, hq * 4:(hq + 1) * 4, :]
                    )
                wv_sb = qkvw.tile([128, HT, 128], dt.bfloat16)
                cc_sb = qkvw.tile([128, T], dt.bfloat16)
                ss_sb = qkvw.tile([128, T], dt.bfloat16)

                # q0,q1,k feature-major with weight-reuse: lhsT held over 4 t-tiles
                for th in range(2):                # halves of the 8 t-tiles
                    x_tiles = []
                    for i in range(4):
                        x_sb = xp.tile([128, HT, 512], dt.bfloat16, name=f"x{i}", bufs=(2 if i <= 1 else 1))
                        for hq in range(4):  # split so MMs can start on early ht tiles
                            nc.sync.dma_start(
                                x_sb[:, hq * 4:(hq + 1) * 4, :],
                                xT[:, hq * 4:(hq + 1) * 4,
                                   (th * 4 + i) * 512:(th * 4 + i + 1) * 512],
                            )
                        x_tiles.append(x_sb)
                    if th == 0:
                        # tables/weights needed later — after the x stream kickoff
                        nc.sync.dma_start(wv_sb[:], wv[:])
                        nc.sync.dma_start(cc_sb[:], cc[:])
                        nc.sync.dma_start(ss_sb[:], ss[:])
                        nc.sync.dma_start(msk_sb[:], msk[:])
                        nc.sync.dma_start(ones_sb[:], ones128[:])
                        nc.sync.dma_start(wo_sb[:], wo[:])
                    for ft in range(3):
                        pss = [
                            qkps.tile([128, 512], dt.float32, name=f"qk{i}", bufs=1)
                            for i in range(4)
                        ]
                        for ht in range(HT):
                            for i in range(4):
                                nc.tensor.matmul(
                                    pss[i][:],
                                    lhsT=wqk_sb[:, ht, ft * 128:(ft + 1) * 128],
                                    rhs=x_tiles[i][:, ht, :],
                                    start=(ht == 0),
                                    stop=(ht == HT - 1),
                                )
                        for i in range(4):
                            tt = th * 4 + i
                            tsl = bass.ts(tt, 512)
                            ps = pss[i]
                            sbq = rp.tile([128, 512], dt.bfloat16)
                            nc.scalar.copy(sbq[:], ps[:])
                            tmp = rp.tile([128, 512], dt.bfloat16)
                            nc.gpsimd.dma_start(tmp[0:64, :], sbq[64:128, :])
                            nc.gpsimd.dma_start(tmp[64:128, :], sbq[0:64, :])
                            qcc = rp.tile([128, 512], dt.bfloat16)
                            nc.vector.tensor_tensor(qcc[:], sbq[:], cc_sb[:, tsl], mult)
                            qss = rp.tile([128, 512], dt.bfloat16)
                            nc.vector.tensor_tensor(qss[:], tmp[:], ss_sb[:, tsl], mult)
                            if ft < 2:
                                dst = qcat[:, ft * T + tt * 512: ft * T + (tt + 1) * 512]
                            else:
                                dst = kT[:, tsl]
                            nc.vector.tensor_tensor(dst, qcc[:], qss[:], add)
                    # token-major v for this half
                    for i in range(4):
                        for st in range(4):
                            psv = vps.tile([128, 128], dt.float32)
                            for ht in range(HT):
                                nc.tensor.matmul(
                                    psv[:],
                                    lhsT=x_tiles[i][:, ht, st * 128:(st + 1) * 128],
                                    rhs=wv_sb[:, ht, :],
                                    start=(ht == 0),
                                    stop=(ht == HT - 1),
                                )
                            t128 = (th * 4 + i) * 4 + st
                            nc.scalar.copy(vsb[:, t128 * 128:(t128 + 1) * 128], psv[:])

            # ---------------- attention + AllGather + W_o ----------------
            with (
                tc.tile_pool(name="spool", bufs=4, space="PSUM") as spool,
                tc.tile_pool(name="pvp", bufs=1, space="PSUM") as pvp,
                tc.tile_pool(name="denp", bufs=1, space="PSUM") as denp,
                tc.tile_pool(name="wops", bufs=1, space="PSUM") as wops,
                tc.tile_pool(name="probs", bufs=6) as probs,
                tc.tile_pool(name="smallp", bufs=4) as smallp,
                tc.tile_pool(name="ap", bufs=3) as apool,
                tc.tile_pool(name="wosb", bufs=2) as wosb,
                tc.tile_pool(name="outp", bufs=2) as outp,
            ):
                def emit_wo(qt, ag_out):
                    # W_o for one chunk's tokens (cb = batch); deferred one chunk
                    # so the PE instruction stream never waits on a fresh gather.
                    for cb in range(2):
                        asb = wosb.tile([128, HT, 512], dt.bfloat16)
                        for dtt in range(HT):
                            nc.sync.dma_start(
                                asb[:, dtt, :], ag_out[dtt, :, cb * 512:(cb + 1) * 512]
                            )
                        for ct in range(2):
                            ps_o = wops.tile([128, 512], dt.float32)
                            for dtt in range(HT):
                                nc.tensor.matmul(
                                    ps_o[:],
                                    lhsT=wo_sb[:, dtt, ct * 128:(ct + 1) * 128],
                                    rhs=asb[:, dtt, :],
                                    start=(dtt == 0), stop=(dtt == HT - 1),
                                )
                            o_sb = outp.tile([128, 512], dt.float32)
                            nc.scalar.copy(o_sb[:], ps_o[:])
                            nc.sync.dma_start(
                                outT[ct * 128:(ct + 1) * 128, cb * S + qt * 512: cb * S + (qt + 1) * 512],
                                o_sb[:],
                            )

                pending_wo = []
                for j, qt in enumerate(QT_ORDER):
                    attn_chunk = dram.tile([256, 2 * 512], dt.bfloat16, name=f"attnc{j}")
                    for b in range(B):
                        col0 = b * 512
                        kts = 4 * qt + 4
                        ps_pv = [pvp.tile([128, 512], dt.float32, name=f"pv{hl}") for hl in range(2)]
                        acc = [smallp.tile([128, 512], dt.bfloat16, name=f"acc{hl}", bufs=2) for hl in range(2)]
                        for kt in range(kts):
                            r = kt - 4 * qt
                            prob2 = []
                            for hl in range(2):
                                ps_s = spool.tile([128, 512], dt.float32)
                                nc.tensor.matmul(
                                    ps_s[:],
                                    lhsT=kT[:, b * S + kt * 128: b * S + (kt + 1) * 128],
                                    rhs=qcat[:, hl * T + b * S + qt * 512: hl * T + b * S + (qt + 1) * 512],
                                    start=True,
                                    stop=True,
                                )
                                prob = probs.tile([128, 512], dt.bfloat16)
                                if r >= 0:
                                    stg = probs.tile([128, 512], dt.bfloat16, name="stg")
                                    nc.scalar.activation(stg[:], ps_s[:], Exp, scale=SCALE)
                                    nc.vector.tensor_tensor(
                                        prob[:], stg[:], msk_sb[:, r * 512:(r + 1) * 512], mult
                                    )
                                else:
                                    nc.scalar.activation(prob[:], ps_s[:], Exp, scale=SCALE)
                                prob2.append(prob)
                            for hl in range(2):
                                nc.tensor.matmul(
                                    ps_pv[hl][:],
                                    lhsT=vsb[:, (b * HT + kt) * 128: (b * HT + kt + 1) * 128],
                                    rhs=prob2[hl][:],
                                    start=(kt == 0), stop=(kt == kts - 1),
                                )
                            for hl in range(2):
                                # denominator partial sums on DVE (sum over k-tiles)
                                if kt == 0:
                                    nc.vector.tensor_copy(acc[hl][:], prob2[hl][:])
                                else:
                                    nc.vector.tensor_add(acc[hl][:], acc[hl][:], prob2[hl][:])
                        for hl in range(2):
                            # partition-reduce + broadcast denominators in one matmul
                            ps_den = denp.tile([128, 512], dt.float32)
                            nc.tensor.matmul(
                                ps_den[:], lhsT=ones_sb[:], rhs=acc[hl][:],
                                start=True, stop=True,
                            )
                            den_sb = smallp.tile([128, 512], dt.float32)
                            nc.vector.reciprocal_approx_fast(den_sb[:], ps_den[:])
                            attn_sb = apool.tile([128, 512], dt.bfloat16)
                            nc.vector.tensor_tensor(attn_sb[:], ps_pv[hl][:], den_sb[:], mult)
                            nc.sync.dma_start(
                                attn_chunk[hl * 128:(hl + 1) * 128, col0:col0 + 512],
                                attn_sb[:],
                            )
                    ag_out = dram.tile(
                        [HT, 128, 2 * 512], dt.bfloat16,
                        addr_space="Shared", name=f"agout{j}",
                    )
                    nc.gpsimd.collective_compute(
                        "AllGather", mybir.AluOpType.bypass, replica_groups=RG8,
                        ins=[attn_chunk.opt()], outs=[ag_out.opt()],
                    )
                    pending_wo.append((qt, ag_out))
                    if len(pending_wo) > 2:
                        emit_wo(*pending_wo.pop(0))
                for w in pending_wo:
                    emit_wo(*w)
    nc.compile()
    return nc


# host-side input prep ------------------------------------------------------

_PERM = np.concatenate([np.arange(0, HD, 2), np.arange(1, HD, 2)])


def _rope_tables():
    freq = 1.0 / (10000.0 ** (np.arange(0, HD, 2, dtype=np.float64) / HD))
    pos = np.arange(S, dtype=np.float64)
    ang = np.outer(pos, freq)                       # [S, 64]
    cos = np.cos(ang).T.astype(np.float32)          # [64, S]
    sin = np.sin(ang).T.astype(np.float32)
    cc1 = np.concatenate([cos, cos], 0)             # [128, S]
    ss1 = np.concatenate([-sin, sin], 0)            # [128, S]
    return (np.tile(cc1, (1, B)).astype(BF16), np.tile(ss1, (1, B)).astype(BF16))


def _prep_inputs(x, W_qkv, W_o):
    x = np.asarray(x, dtype=np.float32)
    W_qkv = np.asarray(W_qkv, dtype=np.float32)
    W_o = np.asarray(W_o, dtype=np.float32)

    xx = np.concatenate([x[0], x[1]], axis=0)       # [4096, 2048]
    xTd = np.ascontiguousarray(
        xx.T.reshape(HT, 128, T).transpose(1, 0, 2)
    ).astype(BF16)                                   # [128, HT, 4096]

    cc, ss = _rope_tables()

    mask = np.zeros((128, 4 * 512), dtype=np.float32)
    ii = np.arange(128)[:, None]
    jj = np.arange(512)[None, :]
    for r in range(4):
        mask[:, r * 512:(r + 1) * 512] = (jj >= ii + 128 * r)
    mask = mask.astype(BF16)

    ones128 = np.ones((128, 128), dtype=np.float32).astype(BF16)

    in_maps = []
    for c in range(8):
        kh = c // 2
        qr = W_qkv[256 * c: 256 * (c + 1)]           # rows of q heads 2c,2c+1
        qr = qr.reshape(2, HD, HIDDEN)[:, _PERM, :].reshape(256, HIDDEN)
        kr = W_qkv[HIDDEN + 128 * kh: HIDDEN + 128 * (kh + 1)][_PERM, :]
        vr = W_qkv[HIDDEN + 512 + 128 * kh: HIDDEN + 512 + 128 * (kh + 1)]
        wqkT = np.ascontiguousarray(
            np.concatenate([qr, kr], 0).T.reshape(HT, 128, 384).transpose(1, 0, 2)
        ).astype(BF16)                               # [128, HT, 384]
        wvT = np.ascontiguousarray(
            vr.T.reshape(HT, 128, 128).transpose(1, 0, 2)
        ).astype(BF16)
        woT = np.ascontiguousarray(
            W_o[256 * c: 256 * (c + 1)].T.reshape(HT, 128, 256).transpose(1, 0, 2)
        ).astype(BF16)
        in_maps.append({
            "xT": xTd, "wqk": wqkT, "wv": wvT, "wo": woT,
            "cc": cc, "ss": ss, "msk": mask, "ones128": ones128,
        })
    return in_maps


def kernel(x, W_qkv, W_o):
    global _COMPILED
    if _COMPILED is None:
        _COMPILED = _build()
    nc = _COMPILED
    in_maps = _prep_inputs(x, W_qkv, W_o)
    res = run_bass_kernel_spmd(nc, in_maps, list(range(8)))
    out = np.empty((B, S, HIDDEN), dtype=np.float32)
    for c in range(8):
        oT = res.results[c]["outT"]                  # [256, 4096]
        out[:, :, 256 * c: 256 * (c + 1)] = oT.reshape(256, B, S).transpose(1, 2, 0)
    return out



# revision 2
# speedup vs baseline: 1.0682x; 1.0682x over previous
"""Trainium2 Bass kernel for nn_Attention_4183298146960.

GQA causal attention layer: B=2, S=2048, HIDDEN=2048, 16 q heads / 4 kv heads,
head_dim=128, RoPE (interleaved pairs), causal softmax, output projection.

Sharding (8 cores, SPMD-uniform program):
  core c owns q heads {2c, 2c+1} and kv head c//2, for BOTH batches
  (tokens axis = [batch0 | batch1] = 4096).

  QKV: each core projects its 2 q heads (256 feats) plus HALF of its kv head:
  even cores compute k (with rope), odd cores compute v (identity rope via
  per-core cos=1/sin=0 tables) -- a 2-rank AllGather per batch exchanges the
  [128, 2048] k/v feature panels inside each core pair, saving 2.1 GFLOP/core
  of projection work.  v arrives feature-major and is transposed to
  token-major on the PE (16x [128,128] transposes per batch).

  Output projection is TOKEN-sharded: attention outputs are redistributed
  with one AllToAll per batch (each core sends its [2 heads x 128, token]
  features split into 8 token shards and receives all 16 heads' features for
  its own 256 tokens) -- 8x less collective wire than AllGathering the full
  activation -- then each core computes the FULL 2048 output features for its
  512 tokens with a streamed copy of W_o.

  The whole kernel is software-pipelined over the two batches:
    S1: QKV(b0)            S2: attn(b0) + A2A#1  interleaved with QKV(b1)
    S3: attn(b1) + A2A#2 interleaved with Wo(b0)     S4: Wo(b1)
  Interleaving is done at matmul granularity (generators merged by a driver)
  so ScalarE exp / DVE softmax work overlaps dense PE phases and the PE never
  waits on the activation pipeline.

Layouts on device (partition dim first): feature-major qT/kT [head_dim, tok]
for scores; token-major v [tok, head_dim] for PV; scores computed transposed
[k, q] so softmax needs no max-subtraction and the denominator is a
ones-matrix matmul producing the broadcast denominator directly; probs stay
unnormalized until after PV.  The causal diagonal 512-block is computed at
128-granularity per head (narrowed matmuls) so only the [128,128] diagonal
sub-blocks need masking.
"""

import numpy as np
import ml_dtypes

import concourse.bass as bass
import concourse.mybir as mybir
import concourse.tile as tile
from concourse import bacc
from concourse.bass_utils import run_bass_kernel_spmd

BF16 = ml_dtypes.bfloat16

HEADS = 16
KV_HEADS = 4
HIDDEN = 2048
HD = 128
S = 2048
B = 2
T = B * S                      # 4096 token axis (both batches)
HT = HIDDEN // 128             # 16 hidden tiles
SCALE = 1.0 / float(np.sqrt(HD))
RG8 = [[0, 1, 2, 3, 4, 5, 6, 7]]
RGPAIR = [[0, 1], [2, 3], [4, 5], [6, 7]]

_COMPILED = None


def _build():
    dt = mybir.dt
    f32 = dt.float32
    bf16 = dt.bfloat16
    nc = bacc.Bacc("TRN2", target_bir_lowering=False, debug=False, num_devices=8)

    xT = nc.dram_tensor("xT", [128, HT, T], bf16, kind="ExternalInput")
    wqk = nc.dram_tensor("wqk", [128, HT, 384], bf16, kind="ExternalInput")
    ccq = nc.dram_tensor("ccq", [128, S], bf16, kind="ExternalInput")
    ssq = nc.dram_tensor("ssq", [128, S], bf16, kind="ExternalInput")
    cck = nc.dram_tensor("cck", [128, S], bf16, kind="ExternalInput")
    ssk = nc.dram_tensor("ssk", [128, S], bf16, kind="ExternalInput")
    msk = nc.dram_tensor("msk", [128, 128], bf16, kind="ExternalInput")
    ones128 = nc.dram_tensor("ones128", [128, 128], bf16, kind="ExternalInput")
    ident128 = nc.dram_tensor("ident128", [128, 128], bf16, kind="ExternalInput")
    wo = nc.dram_tensor("wo", [128, 16, 16, 128], bf16, kind="ExternalInput")
    outT = nc.dram_tensor("outT", [128, 16, B, 256], f32, kind="ExternalOutput")

    mult = mybir.AluOpType.mult
    byp = mybir.AluOpType.bypass
    Exp = mybir.ActivationFunctionType.Exp

    with tile.TileContext(nc) as tc:
        with (
            tc.tile_pool(name="const", bufs=1) as constp,
            tc.tile_pool(name="dram", bufs=1, space="DRAM") as dram,
            tc.tile_pool(name="xp", bufs=2) as xp,
            tc.tile_pool(name="rp", bufs=2) as rp,
            tc.tile_pool(name="pp", bufs=4) as pp,
            tc.tile_pool(name="ap2", bufs=2) as ap2,
            tc.tile_pool(name="wop", bufs=2) as wop,
            tc.tile_pool(name="qps", bufs=1, space="PSUM") as qps,
            tc.tile_pool(name="sps", bufs=4, space="PSUM") as sps,
            tc.tile_pool(name="pvps", bufs=1, space="PSUM") as pvps,
        ):
            # ---- persistent SBUF ----
            qcat = constp.tile([128, 8, 2, 512], bf16)      # q feature-major
            kT = constp.tile([128, T], bf16)
            vsb = constp.tile([128, T], bf16)               # token-major v
            wqk_sb = constp.tile([128, HT, 384], bf16)
            ccq_sb = constp.tile([128, S], bf16)
            ssq_sb = constp.tile([128, S], bf16)
            cck_sb = constp.tile([128, S], bf16)
            ssk_sb = constp.tile([128, S], bf16)
            msk_sb = constp.tile([128, 128], bf16)
            ones_sb = constp.tile([128, 128], bf16)
            ident_sb = constp.tile([128, 128], bf16)

            # ---- DRAM scratch ----
            kvl = [dram.tile([128, S], bf16, name=f"kvl{b}") for b in range(B)]
            kvp = [dram.tile([2, 128, S], bf16, name=f"kvp{b}") for b in range(B)]
            a2i = [dram.tile([8, 256, 256], bf16, name=f"a2i{b}") for b in range(B)]
            a2o = [dram.tile([8, 256, 256], bf16, name=f"a2o{b}") for b in range(B)]
            bar_i = dram.tile([128, 64], bf16, name="bar_i")
            bar_o = dram.tile([8, 128, 64], bf16, name="bar_o", addr_space="Shared")

            # dummy collective first: absorbs the multi-core rendezvous
            # barrier (~40us) while the QKV prologue runs.
            nc.gpsimd.dma_start(bar_i[:, :], msk[:, 0:64])
            nc.gpsimd.collective_compute(
                "AllGather", byp, replica_groups=RG8,
                ins=[bar_i.opt()], outs=[bar_o.opt()],
            )

            xtiles = [None] * 4

            def emit_x_load(g):
                xg = xp.tile([128, HT, 1024], bf16, name="xg")
                for hq in range(4):
                    nc.sync.dma_start(
                        xg[:, hq * 4:(hq + 1) * 4, :],
                        xT[:, hq * 4:(hq + 1) * 4, g * 1024:(g + 1) * 1024],
                    )
                xtiles[g] = xg

            # startup DMAs: first x panel, weights, tables
            emit_x_load(0)
            for hq in range(4):
                nc.sync.dma_start(
                    wqk_sb[:, hq * 4:(hq + 1) * 4, :], wqk[:, hq * 4:(hq + 1) * 4, :]
                )
            nc.sync.dma_start(ccq_sb[:], ccq[:])
            nc.sync.dma_start(ssq_sb[:], ssq[:])
            nc.sync.dma_start(cck_sb[:], cck[:])
            nc.sync.dma_start(ssk_sb[:], ssk[:])
            nc.sync.dma_start(msk_sb[:], msk[:])
            nc.sync.dma_start(ones_sb[:], ones128[:])
            nc.sync.dma_start(ident_sb[:], ident128[:])

            def gen_qkv(b):
                """QKV projection + rope for batch b.  Yields at ~1us PE
                quanta so it can be used as interleave filler."""
                for gi in (0, 1):
                    g = 2 * b + gi
                    if g + 1 <= 3:
                        emit_x_load(g + 1)
                    xg = xtiles[g]
                    for ft in (2, 0, 1):
                        fsl = slice(ft * 128, (ft + 1) * 128)
                        qa = qps.tile([128, 512], f32, name="qa")
                        qb = qps.tile([128, 512], f32, name="qb")
                        for ht in range(HT):
                            nc.tensor.matmul(
                                qa[:], lhsT=wqk_sb[:, ht, fsl], rhs=xg[:, ht, 0:512],
                                start=(ht == 0), stop=(ht == HT - 1),
                            )
                            nc.tensor.matmul(
                                qb[:], lhsT=wqk_sb[:, ht, fsl], rhs=xg[:, ht, 512:1024],
                                start=(ht == 0), stop=(ht == HT - 1),
                            )
                            if ht % 2 == 1:
                                yield
                        sbq = rp.tile([128, 1024], bf16, name="sbq")
                        nc.scalar.copy(sbq[:, 0:512], qa[:])
                        nc.scalar.copy(sbq[:, 512:1024], qb[:])
                        tmp = rp.tile([128, 1024], bf16, name="tmp")
                        nc.gpsimd.dma_start(tmp[0:64, :], sbq[64:128, :])
                        nc.gpsimd.dma_start(tmp[64:128, :], sbq[0:64, :])
                        cc_t, ss_t = (cck_sb, ssk_sb) if ft == 2 else (ccq_sb, ssq_sb)
                        tsl = slice(gi * 1024, (gi + 1) * 1024)
                        qcc = rp.tile([128, 1024], bf16, name="qcc")
                        nc.vector.tensor_tensor(qcc[:], sbq[:], cc_t[:, tsl], mult)
                        qss = rp.tile([128, 1024], bf16, name="qss")
                        nc.vector.tensor_tensor(qss[:], tmp[:], ss_t[:, tsl], mult)
                        if ft == 2:
                            kvs = rp.tile([128, 1024], bf16, name="kvs")
                            nc.vector.tensor_tensor(
                                kvs[:], qcc[:], qss[:], mybir.AluOpType.add
                            )
                            nc.sync.dma_start(
                                kvl[b][:, gi * 1024:(gi + 1) * 1024], kvs[:]
                            )
                            if gi == 1:
                                nc.gpsimd.collective_compute(
                                    "AllGather", byp, replica_groups=RGPAIR,
                                    ins=[kvl[b].opt()], outs=[kvp[b].opt()],
                                )
                        else:
                            for s2 in (0, 1):
                                nc.vector.tensor_tensor(
                                    qcat[:, g * 2 + s2, ft, :],
                                    qcc[:, s2 * 512:(s2 + 1) * 512],
                                    qss[:, s2 * 512:(s2 + 1) * 512],
                                    mybir.AluOpType.add,
                                )
                        yield

            def gen_attn(b):
                """Attention for batch b.  Yields the number of filler quanta
                wanted at each point."""
                # k/v panels from the pair exchange
                nc.sync.dma_start(kT[:, b * S:(b + 1) * S], kvp[b][0])
                vfeat = constp.tile([128, S], bf16, name="vfeat", bufs=1)
                nc.sync.dma_start(vfeat[:], kvp[b][1])
                yield 8   # let filler run while the pair AllGather lands
                for tt in range(16):
                    trp = sps.tile([128, 128], bf16, name="sc")
                    nc.tensor.transpose(
                        trp[:], vfeat[:, tt * 128:(tt + 1) * 128], ident_sb[:]
                    )
                    nc.scalar.copy(
                        vsb[:, (b * 16 + tt) * 128:(b * 16 + tt + 1) * 128], trp[:]
                    )
                    if tt % 4 == 3:
                        yield 1
                for qt in (3, 2, 1, 0):
                    qtg = b * 4 + qt
                    pva = pvps.tile([128, 512], f32, name="pva")
                    pvb = pvps.tile([128, 512], f32, name="pvb")
                    pv = (pva, pvb)
                    acc = [
                        ap2.tile([128, 512], bf16, name=f"acc{h}") for h in (0, 1)
                    ]
                    nkt = 4 * qt
                    first = True
                    for kt in range(nkt):       # off-diagonal k tiles
                        ksl = slice(b * S + kt * 128, b * S + (kt + 1) * 128)
                        prs = []
                        for h in (0, 1):
                            sc = sps.tile([128, 512], f32, name="sc")
                            nc.tensor.matmul(
                                sc[:], lhsT=kT[:, ksl], rhs=qcat[:, qtg, h, :],
                                start=True, stop=True,
                            )
                            pr = pp.tile([128, 512], bf16, name="pr")
                            nc.scalar.activation(pr[:], sc[:], Exp, scale=SCALE)
                            prs.append(pr)
                        yield 1
                        vsl = slice((b * 16 + kt) * 128, (b * 16 + kt + 1) * 128)
                        for h in (0, 1):
                            nc.tensor.matmul(
                                pv[h][:], lhsT=vsb[:, vsl], rhs=prs[h][:],
                                start=first, stop=False,
                            )
                        for h in (0, 1):
                            if first:
                                nc.vector.tensor_copy(acc[h][:], prs[h][:])
                            else:
                                nc.vector.tensor_add(acc[h][:], acc[h][:], prs[h][:])
                        first = False
                    for r in range(4):          # diagonal 512-block, narrowed
                        kt = nkt + r
                        w = 512 - 128 * r
                        ksl = slice(b * S + kt * 128, b * S + (kt + 1) * 128)
                        prs = []
                        for h in (0, 1):
                            sc = sps.tile([128, 512], f32, name="sc")
                            nc.tensor.matmul(
                                sc[:, 0:w], lhsT=kT[:, ksl],
                                rhs=qcat[:, qtg, h, 128 * r:512],
                                start=True, stop=True,
                            )
                            pr = pp.tile([128, 512], bf16, name="pr")
                            nc.scalar.activation(pr[:, 0:w], sc[:, 0:w], Exp, scale=SCALE)
                            nc.vector.tensor_tensor(
                                pr[:, 0:128], pr[:, 0:128], msk_sb[:], mult
                            )
                            prs.append(pr)
                        yield 1
                        vsl = slice((b * 16 + kt) * 128, (b * 16 + kt + 1) * 128)
                        for h in (0, 1):
                            nc.tensor.matmul(
                                pv[h][:, 128 * r:512], lhsT=vsb[:, vsl],
                                rhs=prs[h][:, 0:w],
                                start=first, stop=(r == 3),
                            )
                        for h in (0, 1):
                            if first:
                                nc.vector.tensor_copy(acc[h][:], prs[h][:])
                            else:
                                nc.vector.tensor_add(
                                    acc[h][:, 128 * r:512], acc[h][:, 128 * r:512],
                                    prs[h][:, 0:w],
                                )
                        first = False
                    yield 1
                    for h in (0, 1):
                        den_ps = sps.tile([128, 512], f32, name="sc")
                        nc.tensor.matmul(
                            den_ps[:], lhsT=ones_sb[:], rhs=acc[h][:],
                            start=True, stop=True,
                        )
                        den_sb = ap2.tile([128, 512], f32, name="den")
                        nc.vector.reciprocal_approx_fast(den_sb[:], den_ps[:])
                        att = ap2.tile([128, 512], bf16, name="att")
                        nc.vector.tensor_tensor(att[:], pv[h][:], den_sb[:], mult)
                        for s2 in (0, 1):
                            nc.sync.dma_start(
                                a2i[b][2 * qt + s2, h * 128:(h + 1) * 128, :],
                                att[:, s2 * 256:(s2 + 1) * 256],
                            )
                    yield 2
                nc.gpsimd.collective_compute(
                    "AllToAll", byp, replica_groups=RG8,
                    ins=[a2i[b].opt()], outs=[a2o[b].opt()],
                )

            def gen_wo(b):
                """Output projection for this core's 2x256 tokens of batch b."""
                asb = wop.tile([128, 16, 256], bf16, name="asb")
                for dtt in range(16):
                    nc.sync.dma_start(
                        asb[:, dtt, :],
                        a2o[b][dtt // 2, (dtt % 2) * 128:(dtt % 2) * 128 + 128, :],
                    )
                wocs = [None] * 16

                def load_woc(ct):
                    t = wop.tile([128, 16, 128], bf16, name="woc", bufs=3)
                    nc.sync.dma_start(t[:], wo[:, ct, :, :])
                    wocs[ct] = t

                load_woc(0)
                load_woc(1)
                yield
                for ct in range(16):
                    if ct + 2 < 16:
                        load_woc(ct + 2)
                    ps = qps.tile([128, 512], f32, name=("qa" if ct % 2 == 0 else "qb"))
                    for dtt in range(16):
                        nc.tensor.matmul(
                            ps[:, 0:256], lhsT=wocs[ct][:, dtt, :], rhs=asb[:, dtt, :],
                            start=(dtt == 0), stop=(dtt == 15),
                        )
                        if dtt == 7:
                            yield
                    osb = wop.tile([128, 256], f32, name="osb")
                    nc.scalar.copy(osb[:], ps[:, 0:256])
                    nc.sync.dma_start(outT[:, ct, b, :], osb[:])
                    yield

            def drive(lead, filler, skip_first=0):
                """Run `lead`, inserting the requested number of filler quanta
                at each yield point; drain the filler afterwards."""
                budget = -skip_first
                done = False
                for req in lead:
                    budget += req
                    while budget > 0 and not done:
                        try:
                            next(filler)
                        except StopIteration:
                            done = True
                        budget -= 1
                while not done:
                    try:
                        next(filler)
                    except StopIteration:
                        done = True

            # S1: QKV(b0)
            for _ in gen_qkv(0):
                pass
            # S2: attn(b0) + A2A#1, filled with QKV(b1)
            drive(gen_attn(0), gen_qkv(1))
            # S3: attn(b1) + A2A#2, filled with Wo(b0)
            drive(gen_attn(1), gen_wo(0), skip_first=10)
            # S4: Wo(b1)
            for _ in gen_wo(1):
                pass

    nc.compile()
    return nc


# host-side input prep ------------------------------------------------------

_PERM = np.concatenate([np.arange(0, HD, 2), np.arange(1, HD, 2)])


def _rope_tables():
    freq = 1.0 / (10000.0 ** (np.arange(0, HD, 2, dtype=np.float64) / HD))
    pos = np.arange(S, dtype=np.float64)
    ang = np.outer(pos, freq)                       # [S, 64]
    cos = np.cos(ang).T.astype(np.float32)          # [64, S]
    sin = np.sin(ang).T.astype(np.float32)
    ccq = np.concatenate([cos, cos], 0)             # [128, S]
    ssq = np.concatenate([-sin, sin], 0)
    return ccq.astype(BF16), ssq.astype(BF16)


def _prep_inputs(x, W_qkv, W_o):
    x = np.asarray(x, dtype=np.float32)
    W_qkv = np.asarray(W_qkv, dtype=np.float32)
    W_o = np.asarray(W_o, dtype=np.float32)

    xx = np.concatenate([x[0], x[1]], axis=0)       # [4096, 2048]
    xTd = np.ascontiguousarray(
        xx.T.reshape(HT, 128, T).transpose(1, 0, 2)
    ).astype(BF16)                                   # [128, HT, 4096]

    ccq, ssq = _rope_tables()
    cc_one = np.ones((128, S), dtype=np.float32).astype(BF16)
    ss_zero = np.zeros((128, S), dtype=np.float32).astype(BF16)

    ii = np.arange(128)[:, None]
    jj = np.arange(128)[None, :]
    mask = (jj >= ii).astype(np.float32).astype(BF16)
    ones128 = np.ones((128, 128), dtype=np.float32).astype(BF16)
    ident128 = np.eye(128, dtype=np.float32).astype(BF16)

    # wo[p, ct, dtt, m] = W_o[ct*128+m, dtt*128+p]
    woT = np.ascontiguousarray(
        W_o.reshape(16, 128, 16, 128).transpose(3, 0, 2, 1)
    ).astype(BF16)

    in_maps = []
    for c in range(8):
        kh = c // 2
        qr = W_qkv[256 * c: 256 * (c + 1)]           # rows of q heads 2c,2c+1
        qr = qr.reshape(2, HD, HIDDEN)[:, _PERM, :].reshape(256, HIDDEN)
        if c % 2 == 0:
            kvr = W_qkv[HIDDEN + 128 * kh: HIDDEN + 128 * (kh + 1)][_PERM, :]
            cck, ssk = ccq, ssq
        else:
            kvr = W_qkv[HIDDEN + 512 + 128 * kh: HIDDEN + 512 + 128 * (kh + 1)]
            cck, ssk = cc_one, ss_zero
        wqkT = np.ascontiguousarray(
            np.concatenate([qr, kvr], 0).T.reshape(HT, 128, 384).transpose(1, 0, 2)
        ).astype(BF16)                               # [128, HT, 384]
        in_maps.append({
            "xT": xTd, "wqk": wqkT, "wo": woT,
            "ccq": ccq, "ssq": ssq, "cck": cck, "ssk": ssk,
            "msk": mask, "ones128": ones128, "ident128": ident128,
        })
    return in_maps


def kernel(x, W_qkv, W_o):
    global _COMPILED
    if _COMPILED is None:
        _COMPILED = _build()
    nc = _COMPILED
    in_maps = _prep_inputs(x, W_qkv, W_o)
    res = run_bass_kernel_spmd(nc, in_maps, list(range(8)))
    out = np.empty((B, S, HIDDEN), dtype=np.float32)
    for c in range(8):
        oT = res.results[c]["outT"]                  # [128, 16, B, 256]
        out[:, c * 256:(c + 1) * 256, :] = (
            oT.transpose(2, 3, 1, 0).reshape(B, 256, HIDDEN)
        )
    return out


# revision 3
# speedup vs baseline: 1.1073x; 1.0366x over previous
"""Trainium2 Bass kernel for nn_Attention_4183298146960.

GQA causal attention layer: B=2, S=2048, HIDDEN=2048, 16 q heads / 4 kv heads,
head_dim=128, RoPE (interleaved pairs), causal softmax, output projection.

Sharding (8 cores, SPMD-uniform program):
  core c owns q heads {2c, 2c+1} and kv head c//2, for BOTH batches
  (tokens axis = [batch0 | batch1] = 4096).

  QKV: each core projects its 2 q heads (256 feats) plus HALF of its kv head:
  even cores compute k (with rope), odd cores compute v (identity rope via
  per-core cos=1/sin=0 tables) -- a 2-rank AllGather per batch exchanges the
  [128, 2048] k/v feature panels inside each core pair, saving 2.1 GFLOP/core
  of projection work.  kv is projected FIRST so the exchange overlaps the q
  projections.  v arrives feature-major and is transposed to token-major on
  the PE (16x [128,128] transposes per batch).

  Output projection is TOKEN-sharded: attention outputs are redistributed
  with one AllToAll per half-batch (4 total, 128-token shards) so W_o work
  unblocks progressively -- 8x less collective wire than AllGathering the
  full activation.  Each core computes the FULL 2048 output features for its
  4x128 tokens, with the attention panel as the STATIONARY matmul operand
  (activations = lhsT, W_o streams 512-wide), so W_o stays resident in SBUF
  and is loaded exactly once.

  The whole kernel is software-pipelined over the two batches:
    S1: QKV(b0)            S2: attn(b0) + A2As  interleaved with QKV(b1)
    S3: attn(b1) + A2As  interleaved with Wo(b0)     S4: Wo(b1)
  Interleaving is done at matmul granularity (generators merged by a driver)
  so ScalarE exp / DVE softmax work overlaps dense PE phases and the PE never
  waits on the activation pipeline.

Layouts on device (partition dim first): feature-major qT/kT [head_dim, tok]
for scores; token-major v [tok, head_dim] for PV; scores computed transposed
[k, q] so softmax needs no max-subtraction and the denominator is a
ones-matrix matmul producing the broadcast denominator directly; probs stay
unnormalized until after PV.  The causal diagonal 512-block is computed at
128-granularity per head (narrowed matmuls) so only the [128,128] diagonal
sub-blocks need masking.
"""

import numpy as np
import ml_dtypes

import concourse.bass as bass
import concourse.mybir as mybir
import concourse.tile as tile
from concourse import bacc
from concourse.bass_utils import run_bass_kernel_spmd

BF16 = ml_dtypes.bfloat16

HEADS = 16
KV_HEADS = 4
HIDDEN = 2048
HD = 128
S = 2048
B = 2
T = B * S                      # 4096 token axis (both batches)
HT = HIDDEN // 128             # 16 hidden tiles
SCALE = 1.0 / float(np.sqrt(HD))
RG8 = [[0, 1, 2, 3, 4, 5, 6, 7]]
RGPAIR = [[0, 1], [2, 3], [4, 5], [6, 7]]

_COMPILED = None


def _build():
    dt = mybir.dt
    f32 = dt.float32
    bf16 = dt.bfloat16
    nc = bacc.Bacc("TRN2", target_bir_lowering=False, debug=False, num_devices=8)

    xT = nc.dram_tensor("xT", [128, HT, T], bf16, kind="ExternalInput")
    wqk = nc.dram_tensor("wqk", [128, HT, 384], bf16, kind="ExternalInput")
    ccq = nc.dram_tensor("ccq", [128, S], bf16, kind="ExternalInput")
    ssq = nc.dram_tensor("ssq", [128, S], bf16, kind="ExternalInput")
    cck = nc.dram_tensor("cck", [128, S], bf16, kind="ExternalInput")
    ssk = nc.dram_tensor("ssk", [128, S], bf16, kind="ExternalInput")
    msk = nc.dram_tensor("msk", [128, 128], bf16, kind="ExternalInput")
    ones128 = nc.dram_tensor("ones128", [128, 128], bf16, kind="ExternalInput")
    ident128 = nc.dram_tensor("ident128", [128, 128], bf16, kind="ExternalInput")
    wo = nc.dram_tensor("wo", [128, 16, 2048], bf16, kind="ExternalInput")
    outT = nc.dram_tensor("outT", [128, B, 2, 2048], f32, kind="ExternalOutput")

    mult = mybir.AluOpType.mult
    addop = mybir.AluOpType.add
    byp = mybir.AluOpType.bypass
    Exp = mybir.ActivationFunctionType.Exp

    with tile.TileContext(nc) as tc:
        with (
            tc.tile_pool(name="const", bufs=1) as constp,
            tc.tile_pool(name="dram", bufs=1, space="DRAM") as dram,
            tc.tile_pool(name="pp", bufs=4) as pp,
            tc.tile_pool(name="ap2", bufs=2) as ap2,
            tc.tile_pool(name="qps", bufs=1, space="PSUM") as qps,
            tc.tile_pool(name="sps", bufs=4, space="PSUM") as sps,
            tc.tile_pool(name="pvps", bufs=1, space="PSUM") as pvps,
        ):
            # ---- persistent SBUF ----
            qcat = constp.tile([128, 8, 2, 512], bf16)      # q feature-major
            kT = constp.tile([128, T], bf16)
            vsb = constp.tile([128, T], bf16)               # token-major v
            wqk_sb = constp.tile([128, HT, 384], bf16)
            ccq_sb = constp.tile([128, S], bf16)
            ssq_sb = constp.tile([128, S], bf16)
            cck_sb = constp.tile([128, S], bf16)
            ssk_sb = constp.tile([128, S], bf16)
            msk_sb = constp.tile([128, 128], bf16)
            ones_sb = constp.tile([128, 128], bf16)
            ident_sb = constp.tile([128, 128], bf16)

            # ---- DRAM scratch ----
            kvl = [dram.tile([128, S], bf16, name=f"kvl{b}") for b in range(B)]
            kvp = [dram.tile([2, 128, S], bf16, name=f"kvp{b}") for b in range(B)]
            a2i = [[dram.tile([8, 256, 128], bf16, name=f"a2i{b}{h}")
                    for h in range(2)] for b in range(B)]
            a2o = [[dram.tile([8, 256, 128], bf16, name=f"a2o{b}{h}")
                    for h in range(2)] for b in range(B)]

            xtiles = [None] * 4

            def gen_attn(b):
                """Attention for batch b.  Yields the number of filler quanta
                wanted at each point."""
                # k/v panels from the pair exchange
                nc.sync.dma_start(kT[:, b * S:(b + 1) * S], kvp[b][0])
                vfeat = constp.tile([128, S], bf16, name="vfeat", bufs=1)
                nc.sync.dma_start(vfeat[:], kvp[b][1])
                yield 2
                for tt in range(16):
                    trp = sps.tile([128, 128], bf16, name="sc")
                    nc.tensor.transpose(
                        trp[:], vfeat[:, tt * 128:(tt + 1) * 128], ident_sb[:]
                    )
                    nc.scalar.copy(
                        vsb[:, (b * 16 + tt) * 128:(b * 16 + tt + 1) * 128], trp[:]
                    )
                    if tt % 4 == 3:
                        yield 1
                for qt in (3, 2, 1, 0):
                    qtg = b * 4 + qt
                    pva = pvps.tile([128, 512], f32, name="pva")
                    pvb = pvps.tile([128, 512], f32, name="pvb")
                    pv = (pva, pvb)
                    acc = [
                        ap2.tile([128, 512], bf16, name=f"acc{h}") for h in (0, 1)
                    ]
                    nkt = 4 * qt
                    first = True
                    for kt in range(nkt):       # off-diagonal k tiles
                        ksl = slice(b * S + kt * 128, b * S + (kt + 1) * 128)
                        prs = []
                        for h in (0, 1):
                            sc = sps.tile([128, 512], f32, name="sc")
                            nc.tensor.matmul(
                                sc[:], lhsT=kT[:, ksl], rhs=qcat[:, qtg, h, :],
                                start=True, stop=True,
                            )
                            pr = pp.tile([128, 512], bf16, name="pr")
                            nc.scalar.activation(pr[:], sc[:], Exp, scale=SCALE)
                            prs.append(pr)
                        if kt % 2 == 0:
                            yield 1
                        vsl = slice((b * 16 + kt) * 128, (b * 16 + kt + 1) * 128)
                        for h in (0, 1):
                            nc.tensor.matmul(
                                pv[h][:], lhsT=vsb[:, vsl], rhs=prs[h][:],
                                start=first, stop=False,
                            )
                        for h in (0, 1):
                            if first:
                                nc.vector.tensor_copy(acc[h][:], prs[h][:])
                            else:
                                nc.vector.tensor_add(acc[h][:], acc[h][:], prs[h][:])
                        first = False
                    for r in range(4):          # diagonal 512-block, narrowed
                        kt = nkt + r
                        w = 512 - 128 * r
                        ksl = slice(b * S + kt * 128, b * S + (kt + 1) * 128)
                        prs = []
                        for h in (0, 1):
                            sc = sps.tile([128, 512], f32, name="sc")
                            nc.tensor.matmul(
                                sc[:, 0:w], lhsT=kT[:, ksl],
                                rhs=qcat[:, qtg, h, 128 * r:512],
                                start=True, stop=True,
                            )
                            pr = pp.tile([128, 512], bf16, name="pr")
                            nc.scalar.activation(pr[:, 0:w], sc[:, 0:w], Exp, scale=SCALE)
                            nc.vector.tensor_tensor(
                                pr[:, 0:128], pr[:, 0:128], msk_sb[:], mult
                            )
                            prs.append(pr)
                        yield 1
                        vsl = slice((b * 16 + kt) * 128, (b * 16 + kt + 1) * 128)
                        for h in (0, 1):
                            nc.tensor.matmul(
                                pv[h][:, 128 * r:512], lhsT=vsb[:, vsl],
                                rhs=prs[h][:, 0:w],
                                start=first, stop=(r == 3),
                            )
                        for h in (0, 1):
                            if first:
                                nc.vector.tensor_copy(acc[h][:], prs[h][:])
                            else:
                                nc.vector.tensor_add(
                                    acc[h][:, 128 * r:512], acc[h][:, 128 * r:512],
                                    prs[h][:, 0:w],
                                )
                        first = False
                    yield 1
                    for h in (0, 1):
                        den_ps = sps.tile([128, 512], f32, name="sc")
                        nc.tensor.matmul(
                            den_ps[:], lhsT=ones_sb[:], rhs=acc[h][:],
                            start=True, stop=True,
                        )
                        den_sb = ap2.tile([128, 512], f32, name="den")
                        nc.vector.reciprocal_approx_fast(den_sb[:], den_ps[:])
                        att = ap2.tile([128, 512], bf16, name="att")
                        nc.vector.tensor_tensor(att[:], pv[h][:], den_sb[:], mult)
                        for s2 in range(4):
                            nc.sync.dma_start(
                                a2i[b][qt // 2][(qt % 2) * 4 + s2,
                                                h * 128:(h + 1) * 128, :],
                                att[:, s2 * 128:(s2 + 1) * 128],
                            )
                    if qt == 2 or qt == 0:      # half-batch done -> redistribute
                        hh = qt // 2
                        nc.gpsimd.collective_compute(
                            "AllToAll", byp, replica_groups=RG8,
                            ins=[a2i[b][hh].opt()], outs=[a2o[b][hh].opt()],
                        )
                    yield 2

            with tc.tile_pool(name="xp", bufs=2) as xp, \
                 tc.tile_pool(name="rp", bufs=2) as rp:

                def emit_x_load(g):
                    xg = xp.tile([128, HT, 1024], bf16, name="xg")
                    for hq in range(4):
                        nc.sync.dma_start(
                            xg[:, hq * 4:(hq + 1) * 4, :],
                            xT[:, hq * 4:(hq + 1) * 4, g * 1024:(g + 1) * 1024],
                        )
                    xtiles[g] = xg

                # startup DMAs: first two x panels, weights, tables
                emit_x_load(0)
                for hq in range(4):
                    nc.sync.dma_start(
                        wqk_sb[:, hq * 4:(hq + 1) * 4, :],
                        wqk[:, hq * 4:(hq + 1) * 4, :],
                    )
                nc.sync.dma_start(ccq_sb[:], ccq[:])
                nc.sync.dma_start(ssq_sb[:], ssq[:])
                nc.sync.dma_start(cck_sb[:], cck[:])
                nc.sync.dma_start(ssk_sb[:], ssk[:])
                nc.sync.dma_start(msk_sb[:], msk[:])
                nc.sync.dma_start(ones_sb[:], ones128[:])
                nc.sync.dma_start(ident_sb[:], ident128[:])
                emit_x_load(1)

                def do_ft(b, gi, ft):
                    """One [128-feature x 1024-token] projection + rope."""
                    g = 2 * b + gi
                    xg = xtiles[g]
                    fsl = slice(ft * 128, (ft + 1) * 128)
                    qa = qps.tile([128, 512], f32, name="qa")
                    qb = qps.tile([128, 512], f32, name="qb")
                    for ht in range(HT):
                        nc.tensor.matmul(
                            qa[:], lhsT=wqk_sb[:, ht, fsl], rhs=xg[:, ht, 0:512],
                            start=(ht == 0), stop=(ht == HT - 1),
                        )
                        nc.tensor.matmul(
                            qb[:], lhsT=wqk_sb[:, ht, fsl], rhs=xg[:, ht, 512:1024],
                            start=(ht == 0), stop=(ht == HT - 1),
                        )
                        if ht % 2 == 1:
                            yield
                    sbq = rp.tile([128, 1024], bf16, name="sbq")
                    nc.scalar.copy(sbq[:, 0:512], qa[:])
                    nc.scalar.copy(sbq[:, 512:1024], qb[:])
                    tmp = rp.tile([128, 1024], bf16, name="tmp")
                    nc.gpsimd.dma_start(tmp[0:64, :], sbq[64:128, :])
                    nc.gpsimd.dma_start(tmp[64:128, :], sbq[0:64, :])
                    cc_t, ss_t = (cck_sb, ssk_sb) if ft == 2 else (ccq_sb, ssq_sb)
                    tsl = slice(gi * 1024, (gi + 1) * 1024)
                    qcc = rp.tile([128, 1024], bf16, name="qcc")
                    nc.vector.tensor_tensor(qcc[:], sbq[:], cc_t[:, tsl], mult)
                    qss = rp.tile([128, 1024], bf16, name="qss")
                    nc.vector.tensor_tensor(qss[:], tmp[:], ss_t[:, tsl], mult)
                    if ft == 2:
                        kvs = rp.tile([128, 1024], bf16, name="kvs")
                        nc.vector.tensor_tensor(kvs[:], qcc[:], qss[:], addop)
                        nc.sync.dma_start(
                            kvl[b][:, gi * 1024:(gi + 1) * 1024], kvs[:]
                        )
                        if gi == 1:
                            nc.gpsimd.collective_compute(
                                "AllGather", byp, replica_groups=RGPAIR,
                                ins=[kvl[b].opt()], outs=[kvp[b].opt()],
                            )
                    else:
                        for s2 in (0, 1):
                            nc.vector.tensor_tensor(
                                qcat[:, g * 2 + s2, ft, :],
                                qcc[:, s2 * 512:(s2 + 1) * 512],
                                qss[:, s2 * 512:(s2 + 1) * 512],
                                addop,
                            )
                    yield

                def gen_qkv(b):
                    """QKV for batch b: kv features first (both token groups)
                    so the pair exchange overlaps the q projections."""
                    for gi in (0, 1):
                        if b == 1 and gi == 0:
                            emit_x_load(3)
                        yield from do_ft(b, gi, 2)
                    for gi in (0, 1):
                        if b == 0 and gi == 1:
                            emit_x_load(2)
                        for ft in (0, 1):
                            yield from do_ft(b, gi, ft)

                def drive(lead, filler):
                    budget = 0
                    done = False
                    for req in lead:
                        budget += req
                        while budget > 0 and not done:
                            try:
                                next(filler)
                            except StopIteration:
                                done = True
                            budget -= 1
                    while not done:
                        try:
                            next(filler)
                        except StopIteration:
                            done = True

                # S1: QKV(b0)
                for _ in gen_qkv(0):
                    pass
                # S2: attn(b0) + A2As, filled with QKV(b1)
                drive(gen_attn(0), gen_qkv(1))

            # x/rope pools released -> SBUF for the resident W_o panel
            with tc.tile_pool(name="wop", bufs=2) as wop:
                woB = wop.tile([128, HT, 2048], bf16, name="woB", bufs=1)
                for dtt in range(HT):
                    nc.sync.dma_start(woB[:, dtt, :], wo[:, dtt, :])

                def gen_wo(b):
                    """Output projection for this core's 4x128 tokens: the
                    attention panel is the stationary operand, W_o streams."""
                    for hh in (1, 0):
                        asb = wop.tile([128, 16, 128], bf16, name="asb")
                        for dtt in range(16):
                            nc.sync.dma_start(
                                asb[:, dtt, :],
                                a2o[b][hh][dtt // 2,
                                           (dtt % 2) * 128:(dtt % 2) * 128 + 128, :],
                            )
                        yield
                        for ofh in (0, 1):
                            qa = qps.tile([128, 512], f32, name="qa")
                            qb = qps.tile([128, 512], f32, name="qb")
                            for dtt in range(16):
                                o0 = ofh * 1024
                                nc.tensor.matmul(
                                    qa[:], lhsT=asb[:, dtt, :],
                                    rhs=woB[:, dtt, o0:o0 + 512],
                                    start=(dtt == 0), stop=(dtt == 15),
                                )
                                nc.tensor.matmul(
                                    qb[:], lhsT=asb[:, dtt, :],
                                    rhs=woB[:, dtt, o0 + 512:o0 + 1024],
                                    start=(dtt == 0), stop=(dtt == 15),
                                )
                                if dtt % 4 == 3:
                                    yield
                            for k2, ps in ((0, qa), (1, qb)):
                                osb = wop.tile([128, 512], f32, name="osb")
                                nc.scalar.copy(osb[:], ps[:])
                                nc.sync.dma_start(
                                    outT[:, b, hh,
                                         ofh * 1024 + k2 * 512:
                                         ofh * 1024 + (k2 + 1) * 512],
                                    osb[:],
                                )
                            yield

                def drive2(lead, filler):
                    budget = 0
                    done = False
                    for req in lead:
                        budget += req
                        while budget > 0 and not done:
                            try:
                                next(filler)
                            except StopIteration:
                                done = True
                            budget -= 1
                    while not done:
                        try:
                            next(filler)
                        except StopIteration:
                            done = True

                # S3: attn(b1) + A2As, filled with Wo(b0)
                drive2(gen_attn(1), gen_wo(0))
                # S4: Wo(b1)
                for _ in gen_wo(1):
                    pass

    nc.compile()
    return nc


# host-side input prep ------------------------------------------------------

_PERM = np.concatenate([np.arange(0, HD, 2), np.arange(1, HD, 2)])


def _rope_tables():
    freq = 1.0 / (10000.0 ** (np.arange(0, HD, 2, dtype=np.float64) / HD))
    pos = np.arange(S, dtype=np.float64)
    ang = np.outer(pos, freq)                       # [S, 64]
    cos = np.cos(ang).T.astype(np.float32)          # [64, S]
    sin = np.sin(ang).T.astype(np.float32)
    ccq = np.concatenate([cos, cos], 0)             # [128, S]
    ssq = np.concatenate([-sin, sin], 0)
    return ccq.astype(BF16), ssq.astype(BF16)


def _prep_inputs(x, W_qkv, W_o):
    x = np.asarray(x, dtype=np.float32)
    W_qkv = np.asarray(W_qkv, dtype=np.float32)
    W_o = np.asarray(W_o, dtype=np.float32)

    xx = np.concatenate([x[0], x[1]], axis=0)       # [4096, 2048]
    xTd = np.ascontiguousarray(
        xx.T.reshape(HT, 128, T).transpose(1, 0, 2)
    ).astype(BF16)                                   # [128, HT, 4096]

    ccq, ssq = _rope_tables()
    cc_one = np.ones((128, S), dtype=np.float32).astype(BF16)
    ss_zero = np.zeros((128, S), dtype=np.float32).astype(BF16)

    ii = np.arange(128)[:, None]
    jj = np.arange(128)[None, :]
    mask = (jj >= ii).astype(np.float32).astype(BF16)
    ones128 = np.ones((128, 128), dtype=np.float32).astype(BF16)
    ident128 = np.eye(128, dtype=np.float32).astype(BF16)

    # wo[p, dtt, of] = W_o[of, dtt*128+p]
    woT = np.ascontiguousarray(
        W_o.T.reshape(16, 128, 2048).transpose(1, 0, 2)
    ).astype(BF16)                                   # [128, 16, 2048]

    in_maps = []
    for c in range(8):
        kh = c // 2
        qr = W_qkv[256 * c: 256 * (c + 1)]           # rows of q heads 2c,2c+1
        qr = qr.reshape(2, HD, HIDDEN)[:, _PERM, :].reshape(256, HIDDEN)
        if c % 2 == 0:
            kvr = W_qkv[HIDDEN + 128 * kh: HIDDEN + 128 * (kh + 1)][_PERM, :]
            cck, ssk = ccq, ssq
        else:
            kvr = W_qkv[HIDDEN + 512 + 128 * kh: HIDDEN + 512 + 128 * (kh + 1)]
            cck, ssk = cc_one, ss_zero
        wqkT = np.ascontiguousarray(
            np.concatenate([qr, kvr], 0).T.reshape(HT, 128, 384).transpose(1, 0, 2)
        ).astype(BF16)                               # [128, HT, 384]
        in_maps.append({
            "xT": xTd, "wqk": wqkT, "wo": woT,
            "ccq": ccq, "ssq": ssq, "cck": cck, "ssk": ssk,
            "msk": mask, "ones128": ones128, "ident128": ident128,
        })
    return in_maps


def kernel(x, W_qkv, W_o):
    global _COMPILED
    if _COMPILED is None:
        _COMPILED = _build()
    nc = _COMPILED
    in_maps = _prep_inputs(x, W_qkv, W_o)
    res = run_bass_kernel_spmd(nc, in_maps, list(range(8)))
    out = np.empty((B, S, HIDDEN), dtype=np.float32)
    for c in range(8):
        oT = res.results[c]["outT"]                  # [128, B, 2, 2048]
        for b in range(B):
            for hh in range(2):
                out[b, hh * 1024 + c * 128: hh * 1024 + (c + 1) * 128, :] = (
                    oT[:, b, hh, :]
                )
    return out
